# revision 1
# baseline (speedup 1.0000x reference)
"""Causal self-attention (B=2, T=2048, C=1024, 16 heads) on 8 trn2 NeuronCores.

Sharding: tensor-parallel, core c = b*4+g handles batch b (2) x head-group g
(4 heads = 256 channels). Each core computes q/k/v projections for its
channels, causal attention for its 4 heads, and the slice of the output
projection contracting its channels. Host sums the 4 partial outputs per
batch. No cross-core communication on device.
"""

import sys

if "/opt/trn_rl_repo" not in sys.path:
    sys.path.insert(0, "/opt/trn_rl_repo")

import numpy as np

import concourse.bass as bass
import concourse.mybir as mybir
from concourse.bass_utils import run_bass_kernel_spmd
from concourse.tile import TileContext
import concourse.tile_utils as _tile_utils

_tile_utils.max_sbuf_usage = 208 * 1024
from concourse.masks import make_identity
from concourse.vector_clock import ScopedClock

# ---------------------------------------------------------------------------
# Walrus on this image rejects >4 sem waits on a single instruction; the stock
# TileContext tail-drain attaches one wait per active logical processor.
# Split them into standalone wait_ge instructions instead.
def _patched_drain_and_barrier(self, tick_clock, wait_clock):
    probe = mybir.InstNoOp(name="wait_probe", ins=[], outs=[])
    probe.engine = mybir.EngineType.SP
    wait_clock.add_sem_waits(probe, ScopedClock({None: tick_clock.global_clock}))
    waits = (
        list(probe.sync_info.on_wait)
        if probe.sync_info and probe.sync_info.on_wait
        else []
    )
    assert self.sems is not None
    sem_by_num = {s.num: s for s in self.sems.allocated().values()}
    for w in waits:
        assert w.wait_mode == "sem-ge-imm", w
        self.nc.sync.wait_ge(sem_by_num[w.id], w.wait_value)
    self.nc.sync.drain()
    self.nc.all_engine_barrier()
    popped = self.nc._tile_sem_poison_stack.pop()
    assert popped is self._sem_poison
    self.nc.clear_and_free_semaphores(list(self.sems.allocated().values()))
    self.nc.all_engine_barrier()


TileContext._drain_and_barrier = _patched_drain_and_barrier

# The same walrus limit applies to regular instructions (matmul/LDWEIGHTS
# rejects even 2 waits). Split multi-wait instructions: excess waits move to
# single-wait NoOps committed just before on the same engine.
_orig_commit = TileContext._commit_instruction


def _split_commit(self, inst, lazy_reg_writes=True):
    si = inst.sync_info
    if (
        si is not None
        and si.on_wait
        and len(si.on_wait) > 1
        and inst.engine != mybir.EngineType.Unassigned
    ):
        waits = list(si.on_wait)
        for w in waits[:-1]:
            nop = mybir.InstNoOp(
                name=self.nc.get_next_instruction_name(),
                ins=[],
                outs=[],
                engine=inst.engine,
                sync_info=mybir.SyncInfo(on_wait=[w], on_update=[]),
                bass_nofuse=True,
            )
            _orig_commit(self, nop, lazy_reg_writes=False)
        inst.sync_info = mybir.SyncInfo(
            on_wait=[waits[-1]], on_update=list(si.on_update or [])
        )
    _orig_commit(self, inst, lazy_reg_writes)


TileContext._commit_instruction = _split_commit
# ---------------------------------------------------------------------------

N_CORES = 8
B, T, C = 2, 2048, 1024
H = 16
DH = C // H                       # 64
HPC = H // 4                      # 4 heads per core
CS = HPC * DH                     # 256 channels per core
SCALE = 1.0 / np.sqrt(np.float32(C))  # note: sqrt(n_embd), per reference

P = 128                           # partitions
TB = T // P                       # 16 t-blocks of 128
QC = T // 512                     # 4 q-chunks of 512
KO = C // P                       # 8 contraction subtiles for projections

F32 = mybir.dt.float32
BF16 = mybir.dt.bfloat16
# matmul compute dtype: float32r (tf32, full-rate PE) or float32 (exact, 4 cyc/row)
MM_DT = mybir.dt.float32r


def _bf16(a):
    import ml_dtypes
    return np.ascontiguousarray(a, dtype=np.float32).astype(ml_dtypes.bfloat16)


def _tf32_round(a):
    """Round-to-nearest-even fp32 -> tf32 (10-bit mantissa), returned as fp32 bits."""
    if MM_DT == F32:
        return np.ascontiguousarray(a, dtype=np.float32)
    u = np.ascontiguousarray(a, dtype=np.float32).view(np.uint32).astype(np.uint64)
    r = (u + 0x0FFF + ((u >> 13) & 1)) & 0xFFFFE000
    return r.astype(np.uint32).view(np.float32)

TRACE = False        # test.py flips this to profile
TRACE_KWARGS = {}
LAST_RESULT = None   # BassKernelResults of the most recent run

_NC_CACHE = None


def _build_nc():
    nc = bass.Bass()

    xT_d = nc.dram_tensor("xT", [C, T], MM_DT, kind="ExternalInput")
    wqT_d = nc.dram_tensor("wqT", [C, CS], MM_DT, kind="ExternalInput")
    wkT_d = nc.dram_tensor("wkT", [C, CS], MM_DT, kind="ExternalInput")
    wvT_d = nc.dram_tensor("wvT", [C, CS], MM_DT, kind="ExternalInput")
    peT_d = nc.dram_tensor("peT", [CS, T], F32, kind="ExternalInput")
    pen_d = nc.dram_tensor("pen", [T, CS], F32, kind="ExternalInput")
    woT_d = nc.dram_tensor("woT", [CS, C], MM_DT, kind="ExternalInput")
    maskT_d = nc.dram_tensor("maskT", [4, P, 512], F32, kind="ExternalInput")
    out_d = nc.dram_tensor("out", [T, C], F32, kind="ExternalOutput")

    with TileContext(nc) as tc:
        with (
            nc.allow_low_precision(reason="tf32 matmul inputs are rounded on purpose"),
            tc.tile_pool(name="const", bufs=1) as const,
            tc.tile_pool(name="xchunk", bufs=2) as xpool,
            tc.tile_pool(name="yu", bufs=5) as yupool,
            tc.tile_pool(name="pt", bufs=3) as ptpool,
            tc.tile_pool(name="rec", bufs=2) as recpool,
            tc.tile_pool(name="oddtmp", bufs=2) as oddpool,
            tc.tile_pool(name="outp", bufs=2) as outpool,
            tc.tile_pool(name="mm", bufs=2, space="PSUM") as mmps,
            tc.tile_pool(name="ypsum", bufs=2, space="PSUM") as yps,
            tc.tile_pool(name="bcpsum", bufs=2, space="PSUM") as bcps,
            tc.tile_pool(name="scratch", bufs=2, space="DRAM") as drampool,
        ):
            # ---- persistent tiles -------------------------------------------------
            wq_t = const.tile([P, KO, CS], MM_DT, tag="wq")
            wk_t = const.tile([P, KO, CS], MM_DT, tag="wk")
            wv_t = const.tile([P, KO, CS], MM_DT, tag="wv")
            wo_t = const.tile([P, 2, C], MM_DT, tag="wo")
            peT_t = const.tile([P, 2, T], F32, tag="peT")
            pen_t = const.tile([P, TB, CS], F32, tag="pen")
            mask_t = const.tile([P, 4, 512], F32, tag="mask")
            qT_t = const.tile([P, 2, T], BF16, tag="qT")
            kT_t = const.tile([P, 2, T], BF16, tag="kT")
            v_t = const.tile([P, TB, HPC, DH + 1], MM_DT, tag="v")
            yTp_t = const.tile([P, 2, T], MM_DT, tag="yTp")
            ones_t = const.tile([1, DH], MM_DT, tag="ones")

            xT_r = xT_d.rearrange("(o p) t -> p o t", p=P)

            # DMA order = need order: wq/wk + x chunks gate the first matmuls;
            # mask/wo are not needed until attention / output projection.
            nc.sync.dma_start(out=wq_t[:], in_=wqT_d.rearrange("(o p) m -> p o m", p=P))
            nc.sync.dma_start(out=wk_t[:], in_=wkT_d.rearrange("(o p) m -> p o m", p=P))
            x_tiles = []
            for n in range(QC):
                x_t = xpool.tile([P, KO, 512], MM_DT, tag="x", name=f"x_{n}")
                nc.sync.dma_start(
                    out=x_t[:], in_=xT_r[:, :, n * 512:(n + 1) * 512]
                )
                x_tiles.append(x_t)
                if n == 0:
                    nc.sync.dma_start(out=wv_t[:], in_=wvT_d.rearrange("(o p) m -> p o m", p=P))
                    nc.sync.dma_start(out=peT_t[:], in_=peT_d.rearrange("(o p) m -> p o m", p=P))
                    nc.sync.dma_start(out=pen_t[:], in_=pen_d.rearrange("(o p) m -> p o m", p=P))
            nc.sync.dma_start(out=mask_t[:], in_=maskT_d.rearrange("d p m -> p d m"))
            ones_f32 = const.tile([P, TB * HPC], F32, tag="ones_f32")
            nc.gpsimd.memset(ones_f32[:], 1.0)
            nc.vector.tensor_copy(
                out=v_t[:, :, :, DH],
                in_=ones_f32.rearrange("p (a b) -> p a b", a=TB),
            )
            nc.vector.tensor_copy(out=ones_t[:], in_=ones_f32[:1, :DH])

            # ---- phase 1: q/k/v projections --------------------------------------
            for n in range(QC):
                ts = slice(n * 512, (n + 1) * 512)
                x_t = x_tiles[n]
                for (w_t, dst) in ((wq_t, qT_t), (wk_t, kT_t)):
                    for m in range(2):
                        ps_full = mmps.tile([P, 2, 512], F32, tag="mm")
                        ps = ps_full[:, 0, :]
                        for ko in range(KO):
                            nc.tensor.matmul(
                                ps,
                                lhsT=w_t[:, ko, m * P:(m + 1) * P],
                                rhs=x_t[:, ko, :],
                                start=(ko == 0),
                                stop=(ko == KO - 1),
                            )
                        nc.any.tensor_add(
                            out=dst[:, m, ts], in0=ps, in1=peT_t[:, m, ts]
                        )
                for tb4 in range(4):
                    tb = n * 4 + tb4
                    psv_full = mmps.tile([P, 2, 512], F32, tag="mm")
                    psv = psv_full[:, 0, :CS]
                    for ko in range(KO):
                        nc.tensor.matmul(
                            psv,
                            lhsT=x_t[:, ko, tb4 * P:(tb4 + 1) * P],
                            rhs=wv_t[:, ko, :],
                            start=(ko == 0),
                            stop=(ko == KO - 1),
                        )
                    nc.any.tensor_add(
                        out=v_t[:, tb, :, :DH],
                        in0=psv.rearrange("p (h d) -> p h d", h=HPC),
                        in1=pen_t[:, tb, :].rearrange("p (h d) -> p h d", h=HPC),
                    )

            # ---- phase 2: attention ----------------------------------------------
            # k-tiles processed in groups of 2; one exp covers both. In the
            # straddling groups (last four k-tiles of each q-chunk) the exp is
            # narrowed to the non-fully-masked columns and the 0/1 mask
            # multiply covers only columns below the full-valid region.
            for qc in range(QC):
                qs = slice(qc * 512, (qc + 1) * 512)
                yu_tiles = []
                for h in range(HPC):
                    hb = (h % 2) * DH
                    mt = h // 2
                    y_ps = yps.tile([DH + 1, 512], F32, tag="y")
                    nkt = 4 * qc + 4
                    for kg in range(nkt // 2):
                        st_ps = mmps.tile([P, 2, 512], F32, tag="mm")
                        pt_t = ptpool.tile([P, 2, 512], MM_DT, tag="pt")
                        for kcl in range(2):
                            kc = 2 * kg + kcl
                            nc.tensor.matmul(
                                st_ps[:, kcl, :],
                                lhsT=kT_t[hb:hb + DH, mt, kc * P:(kc + 1) * P],
                                rhs=qT_t[hb:hb + DH, mt, qs],
                                start=True,
                                stop=True,
                            )
                        d0 = 2 * kg - 4 * qc  # straddle index of first kc in group
                        if d0 < 0:
                            # fully-causal group: one wide exp, no mask
                            nc.scalar.activation(
                                pt_t.rearrange("p a b -> p (a b)"),
                                st_ps.rearrange("p a b -> p (a b)"),
                                mybir.ActivationFunctionType.Exp,
                                scale=float(SCALE),
                            )
                        else:
                            for kcl in range(2):
                                d = d0 + kcl
                                # columns < 128d are fully masked -> zero them via
                                # the mask product; exp only columns >= 128d
                                lo = 128 * d
                                nc.scalar.activation(
                                    pt_t[:, kcl, lo:],
                                    st_ps[:, kcl, lo:],
                                    mybir.ActivationFunctionType.Exp,
                                    scale=float(SCALE),
                                )
                                # multiply boundary band [lo, lo+128) by 0/1 mask
                                nc.any.tensor_mul(
                                    out=pt_t[:, kcl, lo:lo + P],
                                    in0=pt_t[:, kcl, lo:lo + P],
                                    in1=mask_t[:, d, lo:lo + P],
                                )
                                if d > 0:
                                    # zero the fully-masked columns
                                    nc.any.tensor_scalar_mul(
                                        pt_t[:, kcl, :lo], st_ps[:, kcl, :lo], 0.0
                                    )
                        for kcl in range(2):
                            kc = 2 * kg + kcl
                            nc.tensor.matmul(
                                y_ps[:],
                                lhsT=v_t[:, kc, h, :],
                                rhs=pt_t[:, kcl, :],
                                start=(kc == 0),
                                stop=(kc == nkt - 1),
                            )
                    # drain unnormalized y (+ sums row) to SBUF, freeing psum
                    yu = yupool.tile([DH + 1, 512], F32, tag="yu", name=f"yu_{qc}_{h}")
                    nc.vector.tensor_copy(out=yu[:], in_=y_ps[:])
                    yu_tiles.append(yu)

                # batched normalization for the 4 heads of this q-chunk:
                # gather sums rows into a [128, 16] layout via SBUF->SBUF DMA so
                # the (8-cycle-per-element) reciprocal runs on all lanes, then
                # scatter back to [1, 512] rows for the ones-broadcast matmul.
                sums_dram = drampool.tile([HPC, 512], F32, tag="sums_dram")
                for h in range(HPC):
                    nc.sync.dma_start(
                        out=sums_dram[h:h + 1, :], in_=yu_tiles[h][DH:DH + 1, :]
                    )
                s_resh = recpool.tile([P, HPC, 4], F32, tag="sresh")
                nc.sync.dma_start(
                    out=s_resh[:],
                    in_=sums_dram.rearrange("h (p j) -> p h j", p=P),
                )
                r_resh = recpool.tile([P, HPC, 4], MM_DT, tag="rresh")
                nc.vector.reciprocal(r_resh[:], s_resh[:])
                rec_dram = drampool.tile([HPC, 512], MM_DT, tag="rec_dram")
                nc.sync.dma_start(
                    out=rec_dram.rearrange("h (p j) -> p h j", p=P),
                    in_=r_resh[:],
                )
                rec_all = recpool.tile([1, HPC, 512], MM_DT, tag="recall")
                nc.sync.dma_start(out=rec_all[0:1, :, :], in_=rec_dram[None, :, :])
                for h in range(HPC):
                    mt = h // 2
                    bc_ps = bcps.tile([DH, 512], F32, tag="bc")
                    nc.tensor.matmul(
                        bc_ps[:], lhsT=ones_t[:], rhs=rec_all[0:1, h, :],
                        start=True, stop=True,
                    )
                    if h % 2 == 0:
                        nc.vector.tensor_mul(
                            out=yTp_t[:DH, mt, qs],
                            in0=yu_tiles[h][:DH, :], in1=bc_ps[:],
                        )
                    else:
                        ytn = oddpool.tile([DH, 512], MM_DT, tag="ytn")
                        nc.vector.tensor_mul(
                            out=ytn[:], in0=yu_tiles[h][:DH, :], in1=bc_ps[:]
                        )
                        # partition shift 0-63 -> 64-127 via SBUF->SBUF DMA
                        nc.sync.dma_start(out=yTp_t[DH:2 * DH, mt, qs], in_=ytn[:])

            # ---- phase 3: output projection --------------------------------------
            nc.sync.dma_start(out=wo_t[:], in_=woT_d.rearrange("(o p) m -> p o m", p=P))
            for tb in range(TB):
                tsl = slice(tb * P, (tb + 1) * P)
                for oc in range(2):
                    ps_full = mmps.tile([P, 2, 512], F32, tag="mm")
                    ps = ps_full[:, 0, :]
                    for m in range(2):
                        nc.tensor.matmul(
                            ps,
                            lhsT=yTp_t[:, m, tsl],
                            rhs=wo_t[:, m, oc * 512:(oc + 1) * 512],
                            start=(m == 0),
                            stop=(m == 1),
                        )
                    o_t = outpool.tile([P, 512], F32, tag="out", name=f"o_{tb}_{oc}")
                    nc.any.tensor_copy(out=o_t[:], in_=ps)
                    nc.sync.dma_start(
                        out=out_d[tsl, oc * 512:(oc + 1) * 512], in_=o_t[:]
                    )

    return nc


def _make_masks():
    kp = np.arange(P)[:, None]
    qf = np.arange(512)[None, :]
    m = np.empty((4, P, 512), dtype=np.float32)
    for d in range(4):
        m[d] = (kp <= qf - 128 * d).astype(np.float32)
    return m


def kernel(x, pos_emb, Wq, Wk, Wv, Wo):
    global _NC_CACHE, LAST_RESULT
    x = np.asarray(x, dtype=np.float32)
    pos_emb = np.asarray(pos_emb, dtype=np.float32)
    Wq = np.asarray(Wq, dtype=np.float32)
    Wk = np.asarray(Wk, dtype=np.float32)
    Wv = np.asarray(Wv, dtype=np.float32)
    Wo = np.asarray(Wo, dtype=np.float32)

    if _NC_CACHE is None:
        _NC_CACHE = _build_nc()
    nc = _NC_CACHE

    maskT = _make_masks()
    xT = [_tf32_round(x[b].T) for b in range(B)]
    in_maps = []
    for c in range(N_CORES):
        b, g = divmod(c, 4)
        ch = slice(g * CS, (g + 1) * CS)
        in_maps.append({
            "xT": xT[b],
            "wqT": _tf32_round(Wq[ch, :].T),
            "wkT": _tf32_round(Wk[ch, :].T),
            "wvT": _tf32_round(Wv[ch, :].T),
            "peT": np.ascontiguousarray(pos_emb[:T, ch].T),
            "pen": np.ascontiguousarray(pos_emb[:T, ch]),
            "woT": _tf32_round(Wo[:, ch].T),
            "maskT": maskT,
        })

    res = run_bass_kernel_spmd(
        nc, in_maps, list(range(N_CORES)), trace=TRACE, **TRACE_KWARGS
    )
    LAST_RESULT = res

    out = np.zeros((B, T, C), dtype=np.float32)
    for c in range(N_CORES):
        b = c // 4
        out[b] += res.results[c]["out"]
    return out



# revision 11
# speedup vs baseline: 1.0152x; 1.0152x over previous
"""Causal self-attention (B=2, T=2048, C=1024, 16 heads) on 8 trn2 NeuronCores.

Sharding: tensor-parallel, core c = b*4+g handles batch b (2) x head-group g
(4 heads = 256 channels). Each core computes q/k/v projections for its
channels, causal attention for its 4 heads, and the slice of the output
projection contracting its channels. Host sums the 4 partial outputs per
batch. No cross-core communication on device.

Emission is software-pipelined: projection chunk n+1 and the deferred
output projection are interleaved into attention q-chunk n so the PE
stays back-to-back (max p-state). Causal masking is done by trimming
fully-masked columns out of the QK/AV matmuls and accumulating a -1e30
triangular additive mask into the boundary band via an identity matmul.
"""

import sys

if "/opt/trn_rl_repo" not in sys.path:
    sys.path.insert(0, "/opt/trn_rl_repo")

import numpy as np

import concourse.bass as bass
import concourse.mybir as mybir
from concourse.bass_utils import run_bass_kernel_spmd
from concourse.tile import TileContext
import concourse.tile_utils as _tile_utils

_tile_utils.max_sbuf_usage = 208 * 1024

# ---------------------------------------------------------------------------
# Walrus on this image rejects >4 sem waits on a single instruction; the stock
# TileContext tail-drain attaches one wait per active logical processor.
# Split them into standalone wait_ge instructions instead.
from concourse.vector_clock import ScopedClock


def _patched_drain_and_barrier(self, tick_clock, wait_clock):
    probe = mybir.InstNoOp(name="wait_probe", ins=[], outs=[])
    probe.engine = mybir.EngineType.SP
    wait_clock.add_sem_waits(probe, ScopedClock({None: tick_clock.global_clock}))
    waits = (
        list(probe.sync_info.on_wait)
        if probe.sync_info and probe.sync_info.on_wait
        else []
    )
    assert self.sems is not None
    sem_by_num = {s.num: s for s in self.sems.allocated().values()}
    for w in waits:
        assert w.wait_mode == "sem-ge-imm", w
        self.nc.sync.wait_ge(sem_by_num[w.id], w.wait_value)
    self.nc.sync.drain()
    self.nc.all_engine_barrier()
    popped = self.nc._tile_sem_poison_stack.pop()
    assert popped is self._sem_poison
    self.nc.clear_and_free_semaphores(list(self.sems.allocated().values()))
    self.nc.all_engine_barrier()


TileContext._drain_and_barrier = _patched_drain_and_barrier

# The same walrus limit applies to regular instructions (matmul/LDWEIGHTS
# rejects even 2 waits). Split multi-wait instructions: excess waits move to
# single-wait NoOps committed just before on the same engine.
_orig_commit = TileContext._commit_instruction


def _split_commit(self, inst, lazy_reg_writes=True):
    si = inst.sync_info
    if (
        si is not None
        and si.on_wait
        and len(si.on_wait) > 1
        and inst.engine != mybir.EngineType.Unassigned
    ):
        waits = list(si.on_wait)
        for w in waits[:-1]:
            nop = mybir.InstNoOp(
                name=self.nc.get_next_instruction_name(),
                ins=[],
                outs=[],
                engine=inst.engine,
                sync_info=mybir.SyncInfo(on_wait=[w], on_update=[]),
                bass_nofuse=True,
            )
            _orig_commit(self, nop, lazy_reg_writes=False)
        inst.sync_info = mybir.SyncInfo(
            on_wait=[waits[-1]], on_update=list(si.on_update or [])
        )
    _orig_commit(self, inst, lazy_reg_writes)


TileContext._commit_instruction = _split_commit
# ---------------------------------------------------------------------------

N_CORES = 8
B, T, C = 2, 2048, 1024
H = 16
DH = C // H                       # 64
HPC = H // 4                      # 4 heads per core
CS = HPC * DH                     # 256 channels per core
SCALE = 1.0 / np.sqrt(np.float32(C))  # note: sqrt(n_embd), per reference

P = 128                           # partitions
TB = T // P                       # 16 t-blocks of 128
QC = T // 512                     # 4 q-chunks of 512
KO = C // P                       # 8 contraction subtiles for projections
NEG = -1.0e30                     # additive causal mask

F32 = mybir.dt.float32
BF16 = mybir.dt.bfloat16
# matmul compute dtype: float32r (tf32, full-rate PE) or float32 (exact, 4 cyc/row)
MM_DT = mybir.dt.float32r

EXP = mybir.ActivationFunctionType.Exp


def _tf32_round(a):
    """Round-to-nearest-even fp32 -> tf32 (10-bit mantissa), returned as fp32 bits."""
    if MM_DT == F32:
        return np.ascontiguousarray(a, dtype=np.float32)
    u = np.ascontiguousarray(a, dtype=np.float32).view(np.uint32).astype(np.uint64)
    r = (u + 0x0FFF + ((u >> 13) & 1)) & 0xFFFFE000
    return r.astype(np.uint32).view(np.float32)


TRACE = False        # test.py flips this to profile
TRACE_KWARGS = {}
LAST_RESULT = None   # BassKernelResults of the most recent run

_NC_CACHE = None


def _build_nc():
    nc = bass.Bass()

    xT_d = nc.dram_tensor("xT", [C, T], MM_DT, kind="ExternalInput")
    wqT_d = nc.dram_tensor("wqT", [C, CS], MM_DT, kind="ExternalInput")
    wkT_d = nc.dram_tensor("wkT", [C, CS], MM_DT, kind="ExternalInput")
    wvT_d = nc.dram_tensor("wvT", [C, CS], MM_DT, kind="ExternalInput")
    peT_d = nc.dram_tensor("peT", [CS, T], F32, kind="ExternalInput")
    pen_d = nc.dram_tensor("pen", [T, CS], F32, kind="ExternalInput")
    woT_d = nc.dram_tensor("woT", [CS, C], MM_DT, kind="ExternalInput")
    ident_d = nc.dram_tensor("ident", [P, P], BF16, kind="ExternalInput")
    madd_d = nc.dram_tensor("madd", [P, P], BF16, kind="ExternalInput")
    out_d = nc.dram_tensor("out", [T, C], F32, kind="ExternalOutput")

    with TileContext(nc) as tc:
        with (
            nc.allow_low_precision(reason="bf16/tf32 matmul inputs on purpose"),
            tc.tile_pool(name="const", bufs=1) as const,
            tc.tile_pool(name="xchunk", bufs=2) as xpool,
            tc.tile_pool(name="yu", bufs=5) as yupool,
            tc.tile_pool(name="pt", bufs=3) as ptpool,
            tc.tile_pool(name="rec", bufs=2) as recpool,
            tc.tile_pool(name="oddtmp", bufs=2) as oddpool,
            tc.tile_pool(name="outp", bufs=4) as outpool,
            tc.tile_pool(name="qkps", bufs=2, space="PSUM") as qkps,
            tc.tile_pool(name="projps", bufs=2, space="PSUM") as projps,
            tc.tile_pool(name="ybcps", bufs=2, space="PSUM") as ybcps,
            tc.tile_pool(name="scratch", bufs=2, space="DRAM") as drampool,
        ):
            # ---- persistent tiles -------------------------------------------------
            wq_t = const.tile([P, KO, CS], MM_DT, tag="wq")
            wk_t = const.tile([P, KO, CS], MM_DT, tag="wk")
            wv_t = const.tile([P, KO, CS], MM_DT, tag="wv")
            wo_t = const.tile([P, 2, C], MM_DT, tag="wo")
            peT_t = const.tile([P, 2, T], F32, tag="peT")
            pen_t = const.tile([P, TB, CS], F32, tag="pen")
            qT_t = const.tile([P, 2, T], BF16, tag="qT")
            kT_t = const.tile([P, 2, T], BF16, tag="kT")
            v_t = const.tile([P, TB, HPC, DH + 1], BF16, tag="v")
            yTp_t = const.tile([P, 2, T], MM_DT, tag="yTp")
            ones_t = const.tile([1, DH], MM_DT, tag="ones")
            ident_t = const.tile([P, P], BF16, tag="ident")
            madd_t = const.tile([P, P], BF16, tag="madd")

            xT_r = xT_d.rearrange("(o p) t -> p o t", p=P)
            wq_r = wqT_d.rearrange("(o p) m -> p o m", p=P)
            wk_r = wkT_d.rearrange("(o p) m -> p o m", p=P)
            wv_r = wvT_d.rearrange("(o p) m -> p o m", p=P)
            peT_r = peT_d.rearrange("(o p) m -> p o m", p=P)
            pen_r = pen_d.rearrange("(o p) m -> p o m", p=P)
            wo_r = woT_d.rearrange("(o p) m -> p o m", p=P)

            # ---- DMA emission: fine-grained, in need order ------------------------
            # chunk 0 critical path first: wq/x0 per-ko pieces interleaved.
            x0 = xpool.tile([P, KO, 512], MM_DT, tag="x", name="x_0")
            for ko in range(KO):
                nc.sync.dma_start(out=wq_t[:, ko, :], in_=wq_r[:, ko, :])
                nc.sync.dma_start(out=x0[:, ko, :], in_=xT_r[:, ko, 0:512])
            nc.sync.dma_start(out=peT_t[:, 0, 0:512], in_=peT_r[:, 0, 0:512])
            nc.sync.dma_start(out=peT_t[:, 1, 0:512], in_=peT_r[:, 1, 0:512])
            for ko in range(KO):
                nc.sync.dma_start(out=wk_t[:, ko, :], in_=wk_r[:, ko, :])
            for ko in range(KO):
                nc.sync.dma_start(out=wv_t[:, ko, :], in_=wv_r[:, ko, :])
            nc.sync.dma_start(out=pen_t[:, 0:4, :], in_=pen_r[:, 0:4, :])
            x_tiles = [x0]
            for n in range(1, QC):
                x_t = xpool.tile([P, KO, 512], MM_DT, tag="x", name=f"x_{n}")
                for ko in range(KO):
                    nc.sync.dma_start(
                        out=x_t[:, ko, :], in_=xT_r[:, ko, n * 512:(n + 1) * 512]
                    )
                nc.sync.dma_start(
                    out=peT_t[:, 0, n * 512:(n + 1) * 512],
                    in_=peT_r[:, 0, n * 512:(n + 1) * 512],
                )
                nc.sync.dma_start(
                    out=peT_t[:, 1, n * 512:(n + 1) * 512],
                    in_=peT_r[:, 1, n * 512:(n + 1) * 512],
                )
                nc.sync.dma_start(
                    out=pen_t[:, 4 * n:4 * n + 4, :], in_=pen_r[:, 4 * n:4 * n + 4, :]
                )
                x_tiles.append(x_t)
            nc.sync.dma_start(out=wo_t[:, 0, :], in_=wo_r[:, 0, :])
            nc.sync.dma_start(out=wo_t[:, 1, :], in_=wo_r[:, 1, :])
            nc.sync.dma_start(out=ident_t[:], in_=ident_d[:, :])
            nc.sync.dma_start(out=madd_t[:], in_=madd_d[:, :])

            # ---- constants via gpsimd --------------------------------------------
            ones_f32 = const.tile([P, TB * HPC], F32, tag="ones_f32")
            nc.gpsimd.memset(ones_f32[:], 1.0)
            nc.gpsimd.tensor_copy(
                out=v_t[:, :, :, DH],
                in_=ones_f32.rearrange("p (a b) -> p a b", a=TB),
            )
            nc.vector.tensor_copy(out=ones_t[:], in_=ones_f32[:1, :DH])

            # ---- generator: q/k/v projections for one 512-chunk ------------------
            def g_proj_chunk(n):
                ts = slice(n * 512, (n + 1) * 512)
                x_t = x_tiles[n]
                for (w_t, dst) in ((wq_t, qT_t), (wk_t, kT_t)):
                    for m in range(2):
                        ps = projps.tile([P, 512], F32, tag="proj")
                        for ko in range(KO):
                            nc.tensor.matmul(
                                ps,
                                lhsT=w_t[:, ko, m * P:(m + 1) * P],
                                rhs=x_t[:, ko, :],
                                start=(ko == 0),
                                stop=(ko == KO - 1),
                            )
                        nc.vector.tensor_add(
                            out=dst[:, m, ts], in0=ps, in1=peT_t[:, m, ts]
                        )
                        yield
                for tb4 in range(4):
                    tb = n * 4 + tb4
                    ps = projps.tile([P, 512], F32, tag="proj")
                    psv = ps[:, :CS]
                    for ko in range(KO):
                        nc.tensor.matmul(
                            psv,
                            lhsT=x_t[:, ko, tb4 * P:(tb4 + 1) * P],
                            rhs=wv_t[:, ko, :],
                            start=(ko == 0),
                            stop=(ko == KO - 1),
                        )
                    nc.vector.tensor_add(
                        out=v_t[:, tb, :, :DH],
                        in0=psv.rearrange("p (h d) -> p h d", h=HPC),
                        in1=pen_t[:, tb, :].rearrange("p (h d) -> p h d", h=HPC),
                    )
                    yield

            # ---- generator: output projection for one q-chunk --------------------
            def g_outproj(qc):
                # gpsimd cannot touch PSUM; scalar is busy with exp until the
                # tail, so qc<3 copies go to vector only
                copy_engines = [nc.vector, nc.scalar] if qc == 3 else [nc.vector]
                for i, (tb, oc) in enumerate(
                    [(tb, oc) for tb in range(qc * 4, qc * 4 + 4) for oc in range(2)]
                ):
                    tsl = slice(tb * P, (tb + 1) * P)
                    ps = projps.tile([P, 512], F32, tag="proj")
                    for m in range(2):
                        nc.tensor.matmul(
                            ps,
                            lhsT=yTp_t[:, m, tsl],
                            rhs=wo_t[:, m, oc * 512:(oc + 1) * 512],
                            start=(m == 0),
                            stop=(m == 1),
                        )
                    o_t = outpool.tile([P, 512], F32, tag="out", name=f"o_{tb}_{oc}")
                    eng = copy_engines[i % len(copy_engines)]
                    if eng is nc.scalar:
                        eng.copy(out=o_t[:], in_=ps)
                    else:
                        eng.tensor_copy(out=o_t[:], in_=ps)
                    nc.sync.dma_start(
                        out=out_d[tsl, oc * 512:(oc + 1) * 512], in_=o_t[:]
                    )
                    yield

            # ---- generator: attention + normalization for one q-chunk ------------
            def g_attn(qc):
                qs = slice(qc * 512, (qc + 1) * 512)
                nkt = 4 * qc + 4
                yu_tiles = []
                for h in range(HPC):
                    hb = (h % 2) * DH
                    mt = h // 2
                    y_ps = ybcps.tile([DH + 1, 512], F32, tag="ybc")
                    for kg in range(nkt // 2):
                        st_ps = qkps.tile([P, 2, 512], F32, tag="qk")
                        pt_t = ptpool.tile([P, 2, 512], BF16, tag="pt")
                        d0 = 2 * kg - 4 * qc  # straddle index of first kc in group
                        for kcl in range(2):
                            kc = 2 * kg + kcl
                            d = d0 + kcl
                            lo = 128 * d if d > 0 else 0
                            nc.tensor.matmul(
                                st_ps[:, kcl, lo:],
                                lhsT=kT_t[hb:hb + DH, mt, kc * P:(kc + 1) * P],
                                rhs=qT_t[hb:hb + DH, mt, qc * 512 + lo:(qc + 1) * 512],
                                start=True,
                                stop=(d < 0),
                                skip_group_check=True,
                            )
                            if d >= 0:
                                # accumulate -1e30 above the diagonal of the
                                # boundary band [lo, lo+128) via identity matmul
                                nc.tensor.matmul(
                                    st_ps[:, kcl, lo:lo + P],
                                    lhsT=ident_t[:],
                                    rhs=madd_t[:],
                                    start=False,
                                    stop=True,
                                    skip_group_check=True,
                                )
                        st_flat = st_ps.rearrange("p a b -> p (a b)")
                        pt_flat = pt_t.rearrange("p a b -> p (a b)")
                        if d0 < 0:
                            nc.scalar.activation(pt_flat, st_flat, EXP, scale=float(SCALE))
                        else:
                            # one exp covering both halves from the first valid
                            # column; the stale gap region is never read by AV
                            s = 128 * d0
                            nc.scalar.activation(
                                pt_flat[:, s:], st_flat[:, s:], EXP, scale=float(SCALE)
                            )
                        for kcl in range(2):
                            kc = 2 * kg + kcl
                            d = d0 + kcl
                            lo = 128 * d if d > 0 else 0
                            nc.tensor.matmul(
                                y_ps[:, lo:],
                                lhsT=v_t[:, kc, h, :],
                                rhs=pt_t[:, kcl, lo:],
                                start=(kc == 0),
                                stop=(kc == nkt - 1),
                                skip_group_check=True,
                            )
                        yield
                    # drain unnormalized y (+ sums row) to SBUF, freeing psum
                    # (gpsimd cannot read PSUM)
                    yu = yupool.tile([DH + 1, 512], F32, tag="yu", name=f"yu_{qc}_{h}")
                    nc.vector.tensor_copy(out=yu[:], in_=y_ps[:])
                    yu_tiles.append(yu)
                    yield

                # batched normalization for the 4 heads of this q-chunk:
                # gather sums rows into a [128, 16] layout via SBUF->DRAM->SBUF
                # DMA so the (8-cycle-per-element) reciprocal runs on all lanes,
                # then scatter back to [1, 512] rows for the ones-broadcast matmul.
                sums_dram = drampool.tile([HPC, 512], F32, tag="sums_dram")
                for h in range(HPC):
                    nc.sync.dma_start(
                        out=sums_dram[h:h + 1, :], in_=yu_tiles[h][DH:DH + 1, :]
                    )
                s_resh = recpool.tile([P, HPC, 4], F32, tag="sresh")
                nc.sync.dma_start(
                    out=s_resh[:],
                    in_=sums_dram.rearrange("h (p j) -> p h j", p=P),
                )
                r_resh = recpool.tile([P, HPC, 4], MM_DT, tag="rresh")
                nc.vector.reciprocal(r_resh[:], s_resh[:])
                rec_dram = drampool.tile([HPC, 512], MM_DT, tag="rec_dram")
                nc.sync.dma_start(
                    out=rec_dram.rearrange("h (p j) -> p h j", p=P),
                    in_=r_resh[:],
                )
                rec_all = recpool.tile([1, HPC, 512], MM_DT, tag="recall")
                nc.sync.dma_start(out=rec_all[0:1, :, :], in_=rec_dram[None, :, :])
                yield
                for h in range(HPC):
                    mt = h // 2
                    bc_ps = ybcps.tile([DH + 1, 512], F32, tag="ybc")
                    nc.tensor.matmul(
                        bc_ps[:DH, :], lhsT=ones_t[:], rhs=rec_all[0:1, h, :],
                        start=True, stop=True, skip_group_check=True,
                    )
                    if h % 2 == 0:
                        nc.vector.tensor_mul(
                            out=yTp_t[:DH, mt, qs],
                            in0=yu_tiles[h][:DH, :], in1=bc_ps[:DH, :],
                        )
                    else:
                        ytn = oddpool.tile([DH, 512], MM_DT, tag="ytn")
                        nc.vector.tensor_mul(
                            out=ytn[:], in0=yu_tiles[h][:DH, :], in1=bc_ps[:DH, :]
                        )
                        # partition shift 0-63 -> 64-127 via SBUF->SBUF DMA
                        nc.sync.dma_start(out=yTp_t[DH:2 * DH, mt, qs], in_=ytn[:])
                    yield

            # ---- software-pipelined emission --------------------------------------
            proj_gens = [g_proj_chunk(n) for n in range(QC)]
            out_gens = [g_outproj(qc) for qc in range(QC)]

            for _ in proj_gens[0]:      # pipeline fill
                pass

            fillers_by_qc = {
                0: [proj_gens[1]],
                1: [proj_gens[2]],
                2: [proj_gens[3]],
                3: out_gens[:3],        # deferred output projections
            }
            for qc in range(QC):
                fillers = list(fillers_by_qc[qc])
                fi = 0
                for _ in g_attn(qc):
                    # pull one filler step (round-robin over filler gens)
                    for _try in range(len(fillers)):
                        g = fillers[fi % len(fillers)] if fillers else None
                        if g is None:
                            break
                        fi += 1
                        try:
                            next(g)
                            break
                        except StopIteration:
                            fillers.remove(g)
                # drain leftovers so chunk qc+1 deps are fully emitted
                for g in fillers:
                    for _ in g:
                        pass
            for _ in out_gens[3]:
                pass

    return nc


def kernel(x, pos_emb, Wq, Wk, Wv, Wo):
    global _NC_CACHE, LAST_RESULT
    x = np.asarray(x, dtype=np.float32)
    pos_emb = np.asarray(pos_emb, dtype=np.float32)
    Wq = np.asarray(Wq, dtype=np.float32)
    Wk = np.asarray(Wk, dtype=np.float32)
    Wv = np.asarray(Wv, dtype=np.float32)
    Wo = np.asarray(Wo, dtype=np.float32)

    if _NC_CACHE is None:
        _NC_CACHE = _build_nc()
    nc = _NC_CACHE

    import ml_dtypes

    ident_np = np.eye(P, dtype=np.float32).astype(ml_dtypes.bfloat16)
    kp = np.arange(P)[:, None]
    jj = np.arange(P)[None, :]
    madd_np = np.where(kp <= jj, 0.0, NEG).astype(np.float32).astype(ml_dtypes.bfloat16)

    xT = [_tf32_round(x[b].T) for b in range(B)]
    in_maps = []
    for c in range(N_CORES):
        b, g = divmod(c, 4)
        ch = slice(g * CS, (g + 1) * CS)
        in_maps.append({
            "xT": xT[b],
            "wqT": _tf32_round(Wq[ch, :].T),
            "wkT": _tf32_round(Wk[ch, :].T),
            "wvT": _tf32_round(Wv[ch, :].T),
            "peT": np.ascontiguousarray(pos_emb[:T, ch].T),
            "pen": np.ascontiguousarray(pos_emb[:T, ch]),
            "woT": _tf32_round(Wo[:, ch].T),
            "ident": ident_np,
            "madd": madd_np,
        })

    res = run_bass_kernel_spmd(
        nc, in_maps, list(range(N_CORES)), trace=TRACE, **TRACE_KWARGS
    )
    LAST_RESULT = res

    out = np.zeros((B, T, C), dtype=np.float32)
    for c in range(N_CORES):
        b = c // 4
        out[b] += res.results[c]["out"]
    return out


# revision 30
# speedup vs baseline: 1.1271x; 1.1103x over previous
"""Causal self-attention (B=2, T=2048, C=1024, 16 heads) on 8 trn2 NeuronCores.

Sharding: tensor-parallel, core c = b*4+g handles batch b (2) x head-group g
(4 heads = 256 channels). Each core computes q/k/v projections for its
channels, causal attention for its 4 heads, and the slice of the output
projection contracting its channels. Host sums the 4 partial outputs per
batch. No cross-core communication on device.

Emission is software-pipelined: projection chunk n+1 and the deferred
output projection are interleaved into attention q-chunk n so the PE
stays back-to-back (max p-state). Causal masking is done by trimming
fully-masked columns out of the QK/AV matmuls and accumulating a -1e30
triangular additive mask into the boundary band via an identity matmul.
"""

import sys

if "/opt/trn_rl_repo" not in sys.path:
    sys.path.insert(0, "/opt/trn_rl_repo")

import numpy as np

import concourse.bass as bass
import concourse.mybir as mybir
from concourse.bass_utils import run_bass_kernel_spmd
from concourse.tile import TileContext
import concourse.tile_utils as _tile_utils

_tile_utils.max_sbuf_usage = 208 * 1024

# ---------------------------------------------------------------------------
# Walrus on this image rejects >4 sem waits on a single instruction; the stock
# TileContext tail-drain attaches one wait per active logical processor.
# Split them into standalone wait_ge instructions instead.
from concourse.vector_clock import ScopedClock


def _patched_drain_and_barrier(self, tick_clock, wait_clock):
    probe = mybir.InstNoOp(name="wait_probe", ins=[], outs=[])
    probe.engine = mybir.EngineType.SP
    wait_clock.add_sem_waits(probe, ScopedClock({None: tick_clock.global_clock}))
    waits = (
        list(probe.sync_info.on_wait)
        if probe.sync_info and probe.sync_info.on_wait
        else []
    )
    assert self.sems is not None
    sem_by_num = {s.num: s for s in self.sems.allocated().values()}
    for w in waits:
        assert w.wait_mode == "sem-ge-imm", w
        self.nc.sync.wait_ge(sem_by_num[w.id], w.wait_value)
    self.nc.sync.drain()
    self.nc.all_engine_barrier()
    popped = self.nc._tile_sem_poison_stack.pop()
    assert popped is self._sem_poison
    self.nc.clear_and_free_semaphores(list(self.sems.allocated().values()))
    self.nc.all_engine_barrier()


TileContext._drain_and_barrier = _patched_drain_and_barrier

# The same walrus limit applies to regular instructions (matmul/LDWEIGHTS
# rejects even 2 waits). Split multi-wait instructions: excess waits move to
# single-wait NoOps committed just before on the same engine.
_orig_commit = TileContext._commit_instruction


def _split_commit(self, inst, lazy_reg_writes=True):
    si = inst.sync_info
    if (
        si is not None
        and si.on_wait
        and len(si.on_wait) > 1
        and inst.engine != mybir.EngineType.Unassigned
    ):
        waits = list(si.on_wait)
        for w in waits[:-1]:
            nop = mybir.InstNoOp(
                name=self.nc.get_next_instruction_name(),
                ins=[],
                outs=[],
                engine=inst.engine,
                sync_info=mybir.SyncInfo(on_wait=[w], on_update=[]),
                bass_nofuse=True,
            )
            _orig_commit(self, nop, lazy_reg_writes=False)
        inst.sync_info = mybir.SyncInfo(
            on_wait=[waits[-1]], on_update=list(si.on_update or [])
        )
    _orig_commit(self, inst, lazy_reg_writes)


TileContext._commit_instruction = _split_commit
# ---------------------------------------------------------------------------

N_CORES = 8
B, T, C = 2, 2048, 1024
H = 16
DH = C // H                       # 64
HPC = H // 4                      # 4 heads per core
CS = HPC * DH                     # 256 channels per core
SCALE = 1.0 / np.sqrt(np.float32(C))  # note: sqrt(n_embd), per reference

P = 128                           # partitions
TB = T // P                       # 16 t-blocks of 128
QC = T // 512                     # 4 q-chunks of 512
KO = C // P                       # 8 contraction subtiles for projections
NEG = -1.0e30                     # additive causal mask
NS = 32                           # replicated sums rows (stream-transpose block)
DS = DH + NS                      # v columns / y partitions incl sums rows

F32 = mybir.dt.float32
BF16 = mybir.dt.bfloat16
# matmul compute dtype: float32r (tf32, full-rate PE) or float32 (exact, 4 cyc/row)
MM_DT = mybir.dt.float32r

EXP = mybir.ActivationFunctionType.Exp


def _tf32_round(a):
    """Round-to-nearest-even fp32 -> tf32 (10-bit mantissa), returned as fp32 bits."""
    if MM_DT == F32:
        return np.ascontiguousarray(a, dtype=np.float32)
    u = np.ascontiguousarray(a, dtype=np.float32).view(np.uint32).astype(np.uint64)
    r = (u + 0x0FFF + ((u >> 13) & 1)) & 0xFFFFE000
    return r.astype(np.uint32).view(np.float32)


TRACE = False        # test.py flips this to profile
TRACE_KWARGS = {}
LAST_RESULT = None   # BassKernelResults of the most recent run

_NC_CACHE = None


def _build_nc():
    nc = bass.Bass()

    xT_d = nc.dram_tensor("xT", [C, T], MM_DT, kind="ExternalInput")
    wqT_d = nc.dram_tensor("wqT", [C, CS], MM_DT, kind="ExternalInput")
    wkT_d = nc.dram_tensor("wkT", [C, CS], MM_DT, kind="ExternalInput")
    wvT_d = nc.dram_tensor("wvT", [C, CS], MM_DT, kind="ExternalInput")
    peT_d = nc.dram_tensor("peT", [CS, T], F32, kind="ExternalInput")
    pen_d = nc.dram_tensor("pen", [T, CS], F32, kind="ExternalInput")
    woT_d = nc.dram_tensor("woT", [CS, C], MM_DT, kind="ExternalInput")
    ident_d = nc.dram_tensor("ident", [P, P], BF16, kind="ExternalInput")
    madd_d = nc.dram_tensor("madd", [P, P], BF16, kind="ExternalInput")
    id32_d = nc.dram_tensor("id32", [NS, NS], BF16, kind="ExternalInput")
    ones32_d = nc.dram_tensor("ones32", [NS, DH], BF16, kind="ExternalInput")
    out_d = nc.dram_tensor("out", [T, C], F32, kind="ExternalOutput")

    with TileContext(nc) as tc:
        with (
            nc.allow_low_precision(reason="bf16/tf32 matmul inputs on purpose"),
            tc.tile_pool(name="const", bufs=1) as const,
            tc.tile_pool(name="xchunk", bufs=2) as xpool,
            tc.tile_pool(name="yu", bufs=5) as yupool,
            tc.tile_pool(name="pt", bufs=3) as ptpool,
            tc.tile_pool(name="rec", bufs=2) as recpool,
            tc.tile_pool(name="oddtmp", bufs=2) as oddpool,
            tc.tile_pool(name="outp", bufs=4) as outpool,
            tc.tile_pool(name="qkps", bufs=2, space="PSUM") as qkps,
            tc.tile_pool(name="projps", bufs=2, space="PSUM") as projps,
            tc.tile_pool(name="ybcps", bufs=2, space="PSUM") as ybcps,
        ):
            # ---- persistent tiles -------------------------------------------------
            wq_t = const.tile([P, KO, CS], MM_DT, tag="wq")
            wk_t = const.tile([P, KO, CS], MM_DT, tag="wk")
            wv_t = const.tile([P, KO, CS], MM_DT, tag="wv")
            wo_t = const.tile([P, 2, C], MM_DT, tag="wo")
            peT_t = const.tile([P, 2, T], F32, tag="peT")
            pen_t = const.tile([P, TB, CS], F32, tag="pen")
            qT_t = const.tile([P, 2, T], BF16, tag="qT")
            kT_t = const.tile([P, 2, T], BF16, tag="kT")
            v_t = const.tile([P, TB, HPC, DS], BF16, tag="v")
            yTp_t = const.tile([P, 2, T], MM_DT, tag="yTp")
            ident_t = const.tile([P, P], BF16, tag="ident")
            madd_t = const.tile([P, P], BF16, tag="madd")
            id32_t = const.tile([NS, NS], BF16, tag="id32")
            ones32_t = const.tile([NS, DH], BF16, tag="ones32")

            xT_r = xT_d.rearrange("(o p) t -> p o t", p=P)
            wq_r = wqT_d.rearrange("(o p) m -> p o m", p=P)
            wk_r = wkT_d.rearrange("(o p) m -> p o m", p=P)
            wv_r = wvT_d.rearrange("(o p) m -> p o m", p=P)
            peT_r = peT_d.rearrange("(o p) m -> p o m", p=P)
            pen_r = pen_d.rearrange("(o p) m -> p o m", p=P)
            wo_r = woT_d.rearrange("(o p) m -> p o m", p=P)

            # ---- DMA emission: fine-grained, in need order ------------------------
            # chunk 0 critical path first: wq/x0 per-ko pieces interleaved.
            x0 = xpool.tile([P, KO, 512], MM_DT, tag="x", name="x_0")
            for ko in range(KO):
                nc.sync.dma_start(out=wq_t[:, ko, :], in_=wq_r[:, ko, :])
                nc.sync.dma_start(out=x0[:, ko, :], in_=xT_r[:, ko, 0:512])
            nc.sync.dma_start(out=peT_t[:, 0, 0:512], in_=peT_r[:, 0, 0:512])
            nc.sync.dma_start(out=peT_t[:, 1, 0:512], in_=peT_r[:, 1, 0:512])
            for ko in range(KO):
                nc.sync.dma_start(out=wk_t[:, ko, :], in_=wk_r[:, ko, :])
            for ko in range(KO):
                nc.sync.dma_start(out=wv_t[:, ko, :], in_=wv_r[:, ko, :])
            nc.sync.dma_start(out=pen_t[:, 0:4, :], in_=pen_r[:, 0:4, :])
            x_tiles = [x0]
            for n in range(1, QC):
                x_t = xpool.tile([P, KO, 512], MM_DT, tag="x", name=f"x_{n}")
                for ko in range(KO):
                    nc.sync.dma_start(
                        out=x_t[:, ko, :], in_=xT_r[:, ko, n * 512:(n + 1) * 512]
                    )
                nc.sync.dma_start(
                    out=peT_t[:, 0, n * 512:(n + 1) * 512],
                    in_=peT_r[:, 0, n * 512:(n + 1) * 512],
                )
                nc.sync.dma_start(
                    out=peT_t[:, 1, n * 512:(n + 1) * 512],
                    in_=peT_r[:, 1, n * 512:(n + 1) * 512],
                )
                nc.sync.dma_start(
                    out=pen_t[:, 4 * n:4 * n + 4, :], in_=pen_r[:, 4 * n:4 * n + 4, :]
                )
                x_tiles.append(x_t)
            nc.sync.dma_start(out=wo_t[:, 0, :], in_=wo_r[:, 0, :])
            nc.sync.dma_start(out=wo_t[:, 1, :], in_=wo_r[:, 1, :])
            nc.sync.dma_start(out=ident_t[:], in_=ident_d[:, :])
            nc.sync.dma_start(out=madd_t[:], in_=madd_d[:, :])
            nc.sync.dma_start(out=id32_t[:], in_=id32_d[:, :])
            nc.sync.dma_start(out=ones32_t[:], in_=ones32_d[:, :])

            # ---- constants via gpsimd --------------------------------------------
            ones_f32 = const.tile([P, TB * HPC * NS], F32, tag="ones_f32")
            nc.gpsimd.memset(ones_f32[:], 1.0)
            nc.gpsimd.tensor_copy(
                out=v_t[:, :, :, DH:],
                in_=ones_f32.rearrange("p (a b c) -> p a b c", a=TB, b=HPC),
            )

            # ---- generator: q/k/v projections for one 512-chunk ------------------
            def g_proj_chunk(n):
                ts = slice(n * 512, (n + 1) * 512)
                x_t = x_tiles[n]
                for (w_t, dst) in ((wq_t, qT_t), (wk_t, kT_t)):
                    for m in range(2):
                        ps = projps.tile([P, 512], F32, tag="proj")
                        for ko in range(KO):
                            nc.tensor.matmul(
                                ps,
                                lhsT=w_t[:, ko, m * P:(m + 1) * P],
                                rhs=x_t[:, ko, :],
                                start=(ko == 0),
                                stop=(ko == KO - 1),
                            )
                        nc.vector.tensor_add(
                            out=dst[:, m, ts], in0=ps, in1=peT_t[:, m, ts]
                        )
                        yield
                for tb4 in range(4):
                    tb = n * 4 + tb4
                    ps = projps.tile([P, 512], F32, tag="proj")
                    psv = ps[:, :CS]
                    for ko in range(KO):
                        nc.tensor.matmul(
                            psv,
                            lhsT=x_t[:, ko, tb4 * P:(tb4 + 1) * P],
                            rhs=wv_t[:, ko, :],
                            start=(ko == 0),
                            stop=(ko == KO - 1),
                        )
                    nc.vector.tensor_add(
                        out=v_t[:, tb, :, :DH],
                        in0=psv.rearrange("p (h d) -> p h d", h=HPC),
                        in1=pen_t[:, tb, :].rearrange("p (h d) -> p h d", h=HPC),
                    )
                    yield

            # ---- generator: output projection for one q-chunk --------------------
            def g_outproj(qc):
                # gpsimd cannot touch PSUM; scalar is busy with exp until the
                # tail, so qc<3 copies go to vector only
                copy_engines = [nc.vector, nc.scalar] if qc == 3 else [nc.vector]
                for i, (tb, oc) in enumerate(
                    [(tb, oc) for tb in range(qc * 4, qc * 4 + 4) for oc in range(2)]
                ):
                    tsl = slice(tb * P, (tb + 1) * P)
                    ps = projps.tile([P, 512], F32, tag="proj")
                    for m in range(2):
                        nc.tensor.matmul(
                            ps,
                            lhsT=yTp_t[:, m, tsl],
                            rhs=wo_t[:, m, oc * 512:(oc + 1) * 512],
                            start=(m == 0),
                            stop=(m == 1),
                        )
                    o_t = outpool.tile([P, 512], F32, tag="out", name=f"o_{tb}_{oc}")
                    eng = copy_engines[i % len(copy_engines)]
                    if eng is nc.scalar:
                        eng.copy(out=o_t[:], in_=ps)
                    else:
                        eng.tensor_copy(out=o_t[:], in_=ps)
                    nc.sync.dma_start(
                        out=out_d[tsl, oc * 512:(oc + 1) * 512], in_=o_t[:]
                    )
                    yield

            # ---- generator: attention + normalization for one q-chunk ------------
            # Normalization is per-head and all-compute-engine (no DMA/SP):
            # the 32 replicated sums rows are block-transposed on the DVE
            # (out[i, 32j+k] = s(32j+i)), reciprocal'd on a strided [32,16]
            # view, expanded to a block-diagonal [32,512] (rhs[i,32j+k] =
            # rec(32j+i)*delta(i,k)), and summed across partitions by a
            # ones-matmul, yielding 1/s broadcast over 64 partitions.
            def g_attn(qc):
                qs = slice(qc * 512, (qc + 1) * 512)
                nkt = 4 * qc + 4

                def norm_head(h, yu):
                    mt = h // 2
                    sT = recpool.tile([NS, 512], F32, tag="sT")
                    nc.vector.transpose(out=sT[:], in_=yu[DH:DS, :])
                    rec_t = recpool.tile([NS, 512 // NS], BF16, tag="rec")
                    nc.vector.reciprocal(rec_t[:], sT[:, 0:512:NS])
                    rhs_sb = recpool.tile([NS, 512], BF16, tag="rhs")
                    nc.vector.tensor_mul(
                        out=rhs_sb.rearrange("p (j k) -> p j k", k=NS),
                        in0=rec_t[:, :, None].to_broadcast((NS, 512 // NS, NS)),
                        in1=id32_t[:, None, :].to_broadcast((NS, 512 // NS, NS)),
                    )
                    bc_ps = ybcps.tile([DS, 512], F32, tag="ybc")
                    nc.tensor.matmul(
                        bc_ps[:DH, :], lhsT=ones32_t[:], rhs=rhs_sb[:],
                        start=True, stop=True, skip_group_check=True,
                    )
                    if h % 2 == 0:
                        nc.vector.tensor_mul(
                            out=yTp_t[:DH, mt, qs],
                            in0=yu[:DH, :], in1=bc_ps[:DH, :],
                        )
                    else:
                        ytn = oddpool.tile([DH, 512], MM_DT, tag="ytn")
                        nc.vector.tensor_mul(
                            out=ytn[:], in0=yu[:DH, :], in1=bc_ps[:DH, :]
                        )
                        # partition shift 0-63 -> 64-127 via SBUF->SBUF DMA
                        nc.sync.dma_start(out=yTp_t[DH:2 * DH, mt, qs], in_=ytn[:])

                for h in range(HPC):
                    hb = (h % 2) * DH
                    mt = h // 2
                    y_ps = ybcps.tile([DS, 512], F32, tag="ybc")
                    for kg in range(nkt // 2):
                        st_ps = qkps.tile([P, 2, 512], F32, tag="qk")
                        pt_t = ptpool.tile([P, 2, 512], BF16, tag="pt")
                        d0 = 2 * kg - 4 * qc  # straddle index of first kc in group
                        for kcl in range(2):
                            kc = 2 * kg + kcl
                            d = d0 + kcl
                            lo = 128 * d if d > 0 else 0
                            nc.tensor.matmul(
                                st_ps[:, kcl, lo:],
                                lhsT=kT_t[hb:hb + DH, mt, kc * P:(kc + 1) * P],
                                rhs=qT_t[hb:hb + DH, mt, qc * 512 + lo:(qc + 1) * 512],
                                start=True,
                                stop=(d < 0),
                                skip_group_check=True,
                            )
                            if d >= 0:
                                # accumulate -1e30 above the diagonal of the
                                # boundary band [lo, lo+128) via identity matmul
                                nc.tensor.matmul(
                                    st_ps[:, kcl, lo:lo + P],
                                    lhsT=ident_t[:],
                                    rhs=madd_t[:],
                                    start=False,
                                    stop=True,
                                    skip_group_check=True,
                                )
                        st_flat = st_ps.rearrange("p a b -> p (a b)")
                        pt_flat = pt_t.rearrange("p a b -> p (a b)")
                        if d0 < 0:
                            nc.scalar.activation(pt_flat, st_flat, EXP, scale=float(SCALE))
                        else:
                            # one exp covering both halves from the first valid
                            # column; the stale gap region is never read by AV
                            s = 128 * d0
                            nc.scalar.activation(
                                pt_flat[:, s:], st_flat[:, s:], EXP, scale=float(SCALE)
                            )
                        for kcl in range(2):
                            kc = 2 * kg + kcl
                            d = d0 + kcl
                            lo = 128 * d if d > 0 else 0
                            nc.tensor.matmul(
                                y_ps[:, lo:],
                                lhsT=v_t[:, kc, h, :],
                                rhs=pt_t[:, kcl, lo:],
                                start=(kc == 0),
                                stop=(kc == nkt - 1),
                                skip_group_check=True,
                            )
                        yield "group"
                    # drain unnormalized y (+ sums rows) to SBUF, freeing psum
                    # (gpsimd cannot read PSUM)
                    yu = yupool.tile([DS, 512], F32, tag="yu", name=f"yu_{qc}_{h}")
                    nc.vector.tensor_copy(out=yu[:], in_=y_ps[:])
                    if h < HPC - 1:
                        norm_head(h, yu)
                        yield "group"
                    else:
                        # last head: let the driver dump reserved fillers over
                        # the norm chain's DMA latency
                        yield "norm"
                        norm_head(h, yu)
                        yield "norm"

            # ---- software-pipelined emission --------------------------------------
            proj_gens = [g_proj_chunk(n) for n in range(QC)]
            out_gens = [g_outproj(qc) for qc in range(QC)]

            for _ in proj_gens[0]:      # pipeline fill
                pass

            fillers_by_qc = {
                0: ([proj_gens[1]], 3),
                1: ([proj_gens[2]], 3),
                2: ([proj_gens[3]], 3),
                3: (out_gens[:3], 6),   # deferred output projections
            }

            def pull(fillers, fi):
                for _try in range(len(fillers)):
                    g = fillers[fi[0] % len(fillers)] if fillers else None
                    if g is None:
                        return False
                    fi[0] += 1
                    try:
                        next(g)
                        return True
                    except StopIteration:
                        fillers.remove(g)
                return False

            for qc in range(QC):
                fillers, reserve = fillers_by_qc[qc]
                fillers = list(fillers)
                fi = [0]
                pulled = 0
                # rough step counts: proj chunks have 8 steps, outproj 8 each
                total_steps = 8 * len(fillers)
                for tok in g_attn(qc):
                    if tok == "norm":
                        # dump everything left over the norm-chain latency
                        while pull(fillers, fi):
                            pass
                    elif total_steps - pulled > reserve:
                        if pull(fillers, fi):
                            pulled += 1
                # drain leftovers so chunk qc+1 deps are fully emitted
                while pull(fillers, fi):
                    pass
            for _ in out_gens[3]:
                pass

    return nc


def kernel(x, pos_emb, Wq, Wk, Wv, Wo):
    global _NC_CACHE, LAST_RESULT
    x = np.asarray(x, dtype=np.float32)
    pos_emb = np.asarray(pos_emb, dtype=np.float32)
    Wq = np.asarray(Wq, dtype=np.float32)
    Wk = np.asarray(Wk, dtype=np.float32)
    Wv = np.asarray(Wv, dtype=np.float32)
    Wo = np.asarray(Wo, dtype=np.float32)

    if _NC_CACHE is None:
        _NC_CACHE = _build_nc()
    nc = _NC_CACHE

    import ml_dtypes

    ident_np = np.eye(P, dtype=np.float32).astype(ml_dtypes.bfloat16)
    kp = np.arange(P)[:, None]
    jj = np.arange(P)[None, :]
    madd_np = np.where(kp <= jj, 0.0, NEG).astype(np.float32).astype(ml_dtypes.bfloat16)
    id32_np = np.eye(NS, dtype=np.float32).astype(ml_dtypes.bfloat16)
    ones32_np = np.ones((NS, DH), dtype=np.float32).astype(ml_dtypes.bfloat16)

    xT = [_tf32_round(x[b].T) for b in range(B)]
    in_maps = []
    for c in range(N_CORES):
        b, g = divmod(c, 4)
        ch = slice(g * CS, (g + 1) * CS)
        in_maps.append({
            "xT": xT[b],
            "wqT": _tf32_round(Wq[ch, :].T),
            "wkT": _tf32_round(Wk[ch, :].T),
            "wvT": _tf32_round(Wv[ch, :].T),
            "peT": np.ascontiguousarray(pos_emb[:T, ch].T),
            "pen": np.ascontiguousarray(pos_emb[:T, ch]),
            "woT": _tf32_round(Wo[:, ch].T),
            "ident": ident_np,
            "madd": madd_np,
            "id32": id32_np,
            "ones32": ones32_np,
        })

    res = run_bass_kernel_spmd(
        nc, in_maps, list(range(N_CORES)), trace=TRACE, **TRACE_KWARGS
    )
    LAST_RESULT = res

    out = np.zeros((B, T, C), dtype=np.float32)
    for c in range(N_CORES):
        b = c // 4
        out[b] += res.results[c]["out"]
    return out


# revision 35
# speedup vs baseline: 1.2186x; 1.0811x over previous
"""Causal self-attention (B=2, T=2048, C=1024, 16 heads) on 8 trn2 NeuronCores.

Sharding: tensor-parallel, core c = b*4+g handles batch b (2) x head-group g
(4 heads = 256 channels). Each core computes q/k/v projections for its
channels, causal attention for its 4 heads, and the slice of the output
projection contracting its channels. Host sums the 4 partial outputs per
batch. No cross-core communication on device.

Emission is software-pipelined: projection chunk n+1 and the deferred
output projection are interleaved into attention q-chunk n so the PE
stays back-to-back (max p-state). Causal masking is done by trimming
fully-masked columns out of the QK/AV matmuls and accumulating a -1e30
triangular additive mask into the boundary band via an identity matmul.
"""

import sys

if "/opt/trn_rl_repo" not in sys.path:
    sys.path.insert(0, "/opt/trn_rl_repo")

import numpy as np

import concourse.bass as bass
import concourse.mybir as mybir
from concourse.bass_utils import run_bass_kernel_spmd
from concourse.tile import TileContext
import concourse.tile_utils as _tile_utils

_tile_utils.max_sbuf_usage = 208 * 1024

# ---------------------------------------------------------------------------
# Walrus on this image rejects >4 sem waits on a single instruction; the stock
# TileContext tail-drain attaches one wait per active logical processor.
# Split them into standalone wait_ge instructions instead.
from concourse.vector_clock import ScopedClock


def _patched_drain_and_barrier(self, tick_clock, wait_clock):
    probe = mybir.InstNoOp(name="wait_probe", ins=[], outs=[])
    probe.engine = mybir.EngineType.SP
    wait_clock.add_sem_waits(probe, ScopedClock({None: tick_clock.global_clock}))
    waits = (
        list(probe.sync_info.on_wait)
        if probe.sync_info and probe.sync_info.on_wait
        else []
    )
    assert self.sems is not None
    sem_by_num = {s.num: s for s in self.sems.allocated().values()}
    for w in waits:
        assert w.wait_mode == "sem-ge-imm", w
        self.nc.sync.wait_ge(sem_by_num[w.id], w.wait_value)
    self.nc.sync.drain()
    self.nc.all_engine_barrier()
    popped = self.nc._tile_sem_poison_stack.pop()
    assert popped is self._sem_poison
    self.nc.clear_and_free_semaphores(list(self.sems.allocated().values()))
    self.nc.all_engine_barrier()


TileContext._drain_and_barrier = _patched_drain_and_barrier

# The same walrus limit applies to regular instructions (matmul/LDWEIGHTS
# rejects even 2 waits). Split multi-wait instructions: excess waits move to
# single-wait NoOps committed just before on the same engine.
_orig_commit = TileContext._commit_instruction


def _split_commit(self, inst, lazy_reg_writes=True):
    si = inst.sync_info
    if (
        si is not None
        and si.on_wait
        and len(si.on_wait) > 1
        and inst.engine != mybir.EngineType.Unassigned
    ):
        waits = list(si.on_wait)
        for w in waits[:-1]:
            nop = mybir.InstNoOp(
                name=self.nc.get_next_instruction_name(),
                ins=[],
                outs=[],
                engine=inst.engine,
                sync_info=mybir.SyncInfo(on_wait=[w], on_update=[]),
                bass_nofuse=True,
            )
            _orig_commit(self, nop, lazy_reg_writes=False)
        inst.sync_info = mybir.SyncInfo(
            on_wait=[waits[-1]], on_update=list(si.on_update or [])
        )
    _orig_commit(self, inst, lazy_reg_writes)


TileContext._commit_instruction = _split_commit
# ---------------------------------------------------------------------------

N_CORES = 8
B, T, C = 2, 2048, 1024
H = 16
DH = C // H                       # 64
HPC = H // 4                      # 4 heads per core
CS = HPC * DH                     # 256 channels per core
SCALE = 1.0 / np.sqrt(np.float32(C))  # note: sqrt(n_embd), per reference

P = 128                           # partitions
TB = T // P                       # 16 t-blocks of 128
QC = T // 512                     # 4 q-chunks of 512
KO = C // P                       # 8 contraction subtiles for projections
NEG = -1.0e30                     # additive causal mask
NS = 32                           # replicated sums rows (stream-transpose block)
DS = DH + NS                      # v columns / y partitions incl sums rows

F32 = mybir.dt.float32
BF16 = mybir.dt.bfloat16
# matmul compute dtype: float32r (tf32, full-rate PE) or float32 (exact, 4 cyc/row)
MM_DT = mybir.dt.float32r

EXP = mybir.ActivationFunctionType.Exp


def _tf32_round(a):
    """Round-to-nearest-even fp32 -> tf32 (10-bit mantissa), returned as fp32 bits."""
    if MM_DT == F32:
        return np.ascontiguousarray(a, dtype=np.float32)
    u = np.ascontiguousarray(a, dtype=np.float32).view(np.uint32).astype(np.uint64)
    r = (u + 0x0FFF + ((u >> 13) & 1)) & 0xFFFFE000
    return r.astype(np.uint32).view(np.float32)


TRACE = False        # test.py flips this to profile
TRACE_KWARGS = {}
LAST_RESULT = None   # BassKernelResults of the most recent run

_NC_CACHE = None


def _build_nc():
    nc = bass.Bass()

    xT_d = nc.dram_tensor("xT", [C, T], MM_DT, kind="ExternalInput")
    wqT_d = nc.dram_tensor("wqT", [C, CS], MM_DT, kind="ExternalInput")
    wkT_d = nc.dram_tensor("wkT", [C, CS], MM_DT, kind="ExternalInput")
    wvT_d = nc.dram_tensor("wvT", [C, CS], MM_DT, kind="ExternalInput")
    peT_d = nc.dram_tensor("peT", [CS, T], F32, kind="ExternalInput")
    pen_d = nc.dram_tensor("pen", [T, CS], F32, kind="ExternalInput")
    woT_d = nc.dram_tensor("woT", [CS, C], MM_DT, kind="ExternalInput")
    ident_d = nc.dram_tensor("ident", [P, P], BF16, kind="ExternalInput")
    madd_d = nc.dram_tensor("madd", [P, P], BF16, kind="ExternalInput")
    id32_d = nc.dram_tensor("id32", [NS, NS], BF16, kind="ExternalInput")
    ones32_d = nc.dram_tensor("ones32", [NS, DH], BF16, kind="ExternalInput")
    out_d = nc.dram_tensor("out", [T, C], F32, kind="ExternalOutput")

    with TileContext(nc) as tc:
        with (
            nc.allow_low_precision(reason="bf16/tf32 matmul inputs on purpose"),
            tc.tile_pool(name="const", bufs=1) as const,
            tc.tile_pool(name="xchunk", bufs=2) as xpool,
            tc.tile_pool(name="yu", bufs=5) as yupool,
            tc.tile_pool(name="pt", bufs=4) as ptpool,
            tc.tile_pool(name="rec", bufs=2) as recpool,
            tc.tile_pool(name="oddtmp", bufs=2) as oddpool,
            tc.tile_pool(name="outp", bufs=4) as outpool,
            tc.tile_pool(name="qkps", bufs=2, space="PSUM") as qkps,
            tc.tile_pool(name="projps", bufs=2, space="PSUM") as projps,
            tc.tile_pool(name="ybcps", bufs=2, space="PSUM") as ybcps,
        ):
            # ---- persistent tiles -------------------------------------------------
            wq_t = const.tile([P, KO, CS], MM_DT, tag="wq")
            wk_t = const.tile([P, KO, CS], MM_DT, tag="wk")
            wv_t = const.tile([P, KO, CS], MM_DT, tag="wv")
            wo_t = const.tile([P, 2, C], MM_DT, tag="wo")
            peT_t = const.tile([P, 2, T], F32, tag="peT")
            pen_t = const.tile([P, TB, CS], F32, tag="pen")
            qT_t = const.tile([P, 2, T], BF16, tag="qT")
            kT_t = const.tile([P, 2, T], BF16, tag="kT")
            v_t = const.tile([P, TB, HPC, DS], BF16, tag="v")
            yTp_t = const.tile([P, 2, T], MM_DT, tag="yTp")
            ident_t = const.tile([P, P], BF16, tag="ident")
            madd_t = const.tile([P, P], BF16, tag="madd")
            id32_t = const.tile([NS, NS], BF16, tag="id32")
            ones32_t = const.tile([NS, DH], BF16, tag="ones32")

            xT_r = xT_d.rearrange("(o p) t -> p o t", p=P)
            wq_r = wqT_d.rearrange("(o p) m -> p o m", p=P)
            wk_r = wkT_d.rearrange("(o p) m -> p o m", p=P)
            wv_r = wvT_d.rearrange("(o p) m -> p o m", p=P)
            peT_r = peT_d.rearrange("(o p) m -> p o m", p=P)
            pen_r = pen_d.rearrange("(o p) m -> p o m", p=P)
            wo_r = woT_d.rearrange("(o p) m -> p o m", p=P)

            # ---- DMA emission: fine-grained, in need order ------------------------
            # chunk 0 critical path first: wq/x0 per-ko pieces interleaved.
            x0 = xpool.tile([P, KO, 512], MM_DT, tag="x", name="x_0")
            for ko in range(KO):
                nc.sync.dma_start(out=wq_t[:, ko, :], in_=wq_r[:, ko, :])
                nc.sync.dma_start(out=x0[:, ko, :], in_=xT_r[:, ko, 0:512])
            nc.sync.dma_start(out=peT_t[:, 0, 0:512], in_=peT_r[:, 0, 0:512])
            nc.sync.dma_start(out=peT_t[:, 1, 0:512], in_=peT_r[:, 1, 0:512])
            for ko in range(KO):
                nc.sync.dma_start(out=wk_t[:, ko, :], in_=wk_r[:, ko, :])
            for ko in range(KO):
                nc.sync.dma_start(out=wv_t[:, ko, :], in_=wv_r[:, ko, :])
            nc.sync.dma_start(out=pen_t[:, 0:4, :], in_=pen_r[:, 0:4, :])
            x_tiles = [x0]
            for n in range(1, QC):
                x_t = xpool.tile([P, KO, 512], MM_DT, tag="x", name=f"x_{n}")
                for ko in range(KO):
                    nc.sync.dma_start(
                        out=x_t[:, ko, :], in_=xT_r[:, ko, n * 512:(n + 1) * 512]
                    )
                nc.sync.dma_start(
                    out=peT_t[:, 0, n * 512:(n + 1) * 512],
                    in_=peT_r[:, 0, n * 512:(n + 1) * 512],
                )
                nc.sync.dma_start(
                    out=peT_t[:, 1, n * 512:(n + 1) * 512],
                    in_=peT_r[:, 1, n * 512:(n + 1) * 512],
                )
                nc.sync.dma_start(
                    out=pen_t[:, 4 * n:4 * n + 4, :], in_=pen_r[:, 4 * n:4 * n + 4, :]
                )
                x_tiles.append(x_t)
            nc.sync.dma_start(out=wo_t[:, 0, :], in_=wo_r[:, 0, :])
            nc.sync.dma_start(out=wo_t[:, 1, :], in_=wo_r[:, 1, :])
            nc.sync.dma_start(out=ident_t[:], in_=ident_d[:, :])
            nc.sync.dma_start(out=madd_t[:], in_=madd_d[:, :])
            nc.sync.dma_start(out=id32_t[:], in_=id32_d[:, :])
            nc.sync.dma_start(out=ones32_t[:], in_=ones32_d[:, :])

            # ---- constants via gpsimd --------------------------------------------
            ones_f32 = const.tile([P, TB * HPC * NS], F32, tag="ones_f32")
            nc.gpsimd.memset(ones_f32[:], 1.0)
            nc.gpsimd.tensor_copy(
                out=v_t[:, :, :, DH:],
                in_=ones_f32.rearrange("p (a b c) -> p a b c", a=TB, b=HPC),
            )

            # ---- generator: q/k/v projections for one 512-chunk ------------------
            def g_proj_chunk(n):
                ts = slice(n * 512, (n + 1) * 512)
                x_t = x_tiles[n]
                for (w_t, dst) in ((wq_t, qT_t), (wk_t, kT_t)):
                    for m in range(2):
                        ps = projps.tile([P, 512], F32, tag="proj")
                        for ko in range(KO):
                            nc.tensor.matmul(
                                ps,
                                lhsT=w_t[:, ko, m * P:(m + 1) * P],
                                rhs=x_t[:, ko, :],
                                start=(ko == 0),
                                stop=(ko == KO - 1),
                            )
                        nc.vector.tensor_add(
                            out=dst[:, m, ts], in0=ps, in1=peT_t[:, m, ts]
                        )
                        yield
                for tb4 in range(4):
                    tb = n * 4 + tb4
                    ps = projps.tile([P, 512], F32, tag="proj")
                    psv = ps[:, :CS]
                    for ko in range(KO):
                        nc.tensor.matmul(
                            psv,
                            lhsT=x_t[:, ko, tb4 * P:(tb4 + 1) * P],
                            rhs=wv_t[:, ko, :],
                            start=(ko == 0),
                            stop=(ko == KO - 1),
                        )
                    nc.vector.tensor_add(
                        out=v_t[:, tb, :, :DH],
                        in0=psv.rearrange("p (h d) -> p h d", h=HPC),
                        in1=pen_t[:, tb, :].rearrange("p (h d) -> p h d", h=HPC),
                    )
                    yield

            # ---- generator: output projection for one q-chunk --------------------
            def g_outproj(qc):
                # gpsimd cannot touch PSUM; scalar is busy with exp until the
                # tail, so qc<3 copies go to vector only
                copy_engines = [nc.vector, nc.scalar] if qc == 3 else [nc.vector]
                for i, (tb, oc) in enumerate(
                    [(tb, oc) for tb in range(qc * 4, qc * 4 + 4) for oc in range(2)]
                ):
                    tsl = slice(tb * P, (tb + 1) * P)
                    ps = projps.tile([P, 512], F32, tag="proj")
                    for m in range(2):
                        nc.tensor.matmul(
                            ps,
                            lhsT=yTp_t[:, m, tsl],
                            rhs=wo_t[:, m, oc * 512:(oc + 1) * 512],
                            start=(m == 0),
                            stop=(m == 1),
                        )
                    o_t = outpool.tile([P, 512], F32, tag="out", name=f"o_{tb}_{oc}")
                    eng = copy_engines[i % len(copy_engines)]
                    if eng is nc.scalar:
                        eng.copy(out=o_t[:], in_=ps)
                    else:
                        eng.tensor_copy(out=o_t[:], in_=ps)
                    nc.sync.dma_start(
                        out=out_d[tsl, oc * 512:(oc + 1) * 512], in_=o_t[:]
                    )
                    yield

            # ---- generator: attention + normalization for one q-chunk ------------
            # Normalization is per-head and all-compute-engine (no DMA/SP):
            # the 32 replicated sums rows are block-transposed on the DVE
            # (out[i, 32j+k] = s(32j+i)), reciprocal'd on a strided [32,16]
            # view, expanded to a block-diagonal [32,512] (rhs[i,32j+k] =
            # rec(32j+i)*delta(i,k)), and summed across partitions by a
            # ones-matmul, yielding 1/s broadcast over 64 partitions.
            def g_attn(qc):
                qs = slice(qc * 512, (qc + 1) * 512)
                nkt = 4 * qc + 4

                def norm_vector_part(h, yu):
                    sT = recpool.tile([NS, 512], F32, tag="sT")
                    nc.vector.transpose(out=sT[:], in_=yu[DH:DS, :])
                    rec_t = recpool.tile([NS, 512 // NS], BF16, tag="rec")
                    nc.vector.reciprocal(rec_t[:], sT[:, 0:512:NS])
                    rhs_sb = recpool.tile([NS, 512], BF16, tag="rhs")
                    nc.vector.tensor_mul(
                        out=rhs_sb.rearrange("p (j k) -> p j k", k=NS),
                        in0=rec_t[:, :, None].to_broadcast((NS, 512 // NS, NS)),
                        in1=id32_t[:, None, :].to_broadcast((NS, 512 // NS, NS)),
                    )
                    return rhs_sb

                def norm_bc_mul(h, yu, rhs_sb):
                    mt = h // 2
                    bc_ps = ybcps.tile([DS, 512], F32, tag="ybc")
                    nc.tensor.matmul(
                        bc_ps[:DH, :], lhsT=ones32_t[:], rhs=rhs_sb[:],
                        start=True, stop=True, skip_group_check=True,
                    )
                    if h % 2 == 0:
                        nc.vector.tensor_mul(
                            out=yTp_t[:DH, mt, qs],
                            in0=yu[:DH, :], in1=bc_ps[:DH, :],
                        )
                    else:
                        ytn = oddpool.tile([DH, 512], MM_DT, tag="ytn")
                        nc.vector.tensor_mul(
                            out=ytn[:], in0=yu[:DH, :], in1=bc_ps[:DH, :]
                        )
                        # partition shift 0-63 -> 64-127 via SBUF->SBUF DMA
                        nc.sync.dma_start(out=yTp_t[DH:2 * DH, mt, qs], in_=ytn[:])

                def norm_head(h, yu):
                    norm_bc_mul(h, yu, norm_vector_part(h, yu))

                # head pairs double the QK->exp->AV dependency distance so the
                # PE stays back-to-back (p-state); the last processed head is
                # even (h2) so the tail avoids the odd partition-shift DMA
                for hp, pair in enumerate(((1, 3), (0, 2))):
                    y_tiles = {}
                    for h in pair:
                        y_tiles[h] = ybcps.tile(
                            [DS, 512], F32, tag="ybc", name=f"y_ps_{qc}_{h}"
                        )
                    for kg in range(nkt // 2):
                        d0 = 2 * kg - 4 * qc  # straddle index of first kc
                        st_tiles = {}
                        pt_tiles = {}
                        for h in pair:
                            hb = (h % 2) * DH
                            mt = h // 2
                            st_ps = qkps.tile([P, 2, 512], F32, tag="qk")
                            pt_t = ptpool.tile([P, 2, 512], BF16, tag="pt")
                            st_tiles[h] = st_ps
                            pt_tiles[h] = pt_t
                            for kcl in range(2):
                                kc = 2 * kg + kcl
                                d = d0 + kcl
                                lo = 128 * d if d > 0 else 0
                                nc.tensor.matmul(
                                    st_ps[:, kcl, lo:],
                                    lhsT=kT_t[hb:hb + DH, mt, kc * P:(kc + 1) * P],
                                    rhs=qT_t[hb:hb + DH, mt, qc * 512 + lo:(qc + 1) * 512],
                                    start=True,
                                    stop=(d < 0),
                                    skip_group_check=True,
                                )
                                if d >= 0:
                                    # accumulate -1e30 above the diagonal of
                                    # the boundary band via identity matmul
                                    nc.tensor.matmul(
                                        st_ps[:, kcl, lo:lo + P],
                                        lhsT=ident_t[:],
                                        rhs=madd_t[:],
                                        start=False,
                                        stop=True,
                                        skip_group_check=True,
                                    )
                        for h in pair:
                            st_flat = st_tiles[h].rearrange("p a b -> p (a b)")
                            pt_flat = pt_tiles[h].rearrange("p a b -> p (a b)")
                            if d0 < 0:
                                nc.scalar.activation(
                                    pt_flat, st_flat, EXP, scale=float(SCALE)
                                )
                            else:
                                # one exp from the first valid column; the
                                # stale gap region is never read by AV
                                s = 128 * d0
                                nc.scalar.activation(
                                    pt_flat[:, s:], st_flat[:, s:], EXP,
                                    scale=float(SCALE),
                                )
                        for h in pair:
                            for kcl in range(2):
                                kc = 2 * kg + kcl
                                d = d0 + kcl
                                lo = 128 * d if d > 0 else 0
                                nc.tensor.matmul(
                                    y_tiles[h][:, lo:],
                                    lhsT=v_t[:, kc, h, :],
                                    rhs=pt_tiles[h][:, kcl, lo:],
                                    start=(kc == 0),
                                    stop=(kc == nkt - 1),
                                    skip_group_check=True,
                                )
                        yield "group"
                    # drain unnormalized y (+ sums rows) to SBUF, freeing psum
                    last = hp == 1
                    yus = {}
                    for h in pair:
                        yu = yupool.tile(
                            [DS, 512], F32, tag="yu", name=f"yu_{qc}_{h}"
                        )
                        nc.vector.tensor_copy(out=yu[:], in_=y_tiles[h][:])
                        yus[h] = yu
                    for i, h in enumerate(pair):
                        if last and i == 1:
                            # final head: emit the vector part, dump reserved
                            # fillers, then the bc+mul so the PE has work
                            # queued ahead of bc while the chain completes
                            rhs_sb = norm_vector_part(h, yus[h])
                            yield "norm"
                            norm_bc_mul(h, yus[h], rhs_sb)
                            yield "group"
                        else:
                            norm_head(h, yus[h])
                            yield "group"

            # ---- software-pipelined emission --------------------------------------
            proj_gens = [g_proj_chunk(n) for n in range(QC)]
            out_gens = [g_outproj(qc) for qc in range(QC)]

            for _ in proj_gens[0]:      # pipeline fill
                pass

            fillers_by_qc = {
                0: ([proj_gens[1]], 3),
                1: ([proj_gens[2]], 3),
                2: ([proj_gens[3]], 3),
                3: (out_gens[:3], 6),   # deferred output projections
            }

            def pull(fillers, fi):
                for _try in range(len(fillers)):
                    g = fillers[fi[0] % len(fillers)] if fillers else None
                    if g is None:
                        return False
                    fi[0] += 1
                    try:
                        next(g)
                        return True
                    except StopIteration:
                        fillers.remove(g)
                return False

            for qc in range(QC):
                fillers, reserve = fillers_by_qc[qc]
                fillers = list(fillers)
                fi = [0]
                pulled = 0
                # rough step counts: proj chunks have 8 steps, outproj 8 each
                total_steps = 8 * len(fillers)
                per_yield = 2 if qc == 3 else 1
                for tok in g_attn(qc):
                    if tok == "norm":
                        # dump everything left over the norm-chain latency
                        while pull(fillers, fi):
                            pass
                    elif total_steps - pulled > reserve:
                        for _ in range(per_yield):
                            if pull(fillers, fi):
                                pulled += 1
                # drain leftovers so chunk qc+1 deps are fully emitted
                while pull(fillers, fi):
                    pass
            for _ in out_gens[3]:
                pass

    return nc


def kernel(x, pos_emb, Wq, Wk, Wv, Wo):
    global _NC_CACHE, LAST_RESULT
    x = np.asarray(x, dtype=np.float32)
    pos_emb = np.asarray(pos_emb, dtype=np.float32)
    Wq = np.asarray(Wq, dtype=np.float32)
    Wk = np.asarray(Wk, dtype=np.float32)
    Wv = np.asarray(Wv, dtype=np.float32)
    Wo = np.asarray(Wo, dtype=np.float32)

    if _NC_CACHE is None:
        _NC_CACHE = _build_nc()
    nc = _NC_CACHE

    import ml_dtypes

    ident_np = np.eye(P, dtype=np.float32).astype(ml_dtypes.bfloat16)
    kp = np.arange(P)[:, None]
    jj = np.arange(P)[None, :]
    madd_np = np.where(kp <= jj, 0.0, NEG).astype(np.float32).astype(ml_dtypes.bfloat16)
    id32_np = np.eye(NS, dtype=np.float32).astype(ml_dtypes.bfloat16)
    ones32_np = np.ones((NS, DH), dtype=np.float32).astype(ml_dtypes.bfloat16)

    xT = [_tf32_round(x[b].T) for b in range(B)]
    in_maps = []
    for c in range(N_CORES):
        b, g = divmod(c, 4)
        ch = slice(g * CS, (g + 1) * CS)
        in_maps.append({
            "xT": xT[b],
            "wqT": _tf32_round(Wq[ch, :].T),
            "wkT": _tf32_round(Wk[ch, :].T),
            "wvT": _tf32_round(Wv[ch, :].T),
            "peT": np.ascontiguousarray(pos_emb[:T, ch].T),
            "pen": np.ascontiguousarray(pos_emb[:T, ch]),
            "woT": _tf32_round(Wo[:, ch].T),
            "ident": ident_np,
            "madd": madd_np,
            "id32": id32_np,
            "ones32": ones32_np,
        })

    res = run_bass_kernel_spmd(
        nc, in_maps, list(range(N_CORES)), trace=TRACE, **TRACE_KWARGS
    )
    LAST_RESULT = res

    out = np.zeros((B, T, C), dtype=np.float32)
    for c in range(N_CORES):
        b = c // 4
        out[b] += res.results[c]["out"]
    return out
